# revision 18
# baseline (speedup 1.0000x reference)
"""Trainium2 Bass kernel for fp8 (E4M3) quantized dense layer with bias.

Computes: out = fp8(x) @ fp8(W) + bias
  x: [32768, 1024] f32, W: [1024, 4096] f32, bias: [4096] f32 -> out [32768, 4096] f32

Sharding: data-parallel over tokens (32768/8 = 4096 tokens per core); W and bias
replicated. No collectives needed; per-core outputs concatenate along tokens.

Per-core pipeline (tokens processed in blocks of 128):
  1. DMA x block [128, 1024] f32 -> SBUF
  2. ACT cast f32 -> fp8e4 (TRN E4M3 == OCP E4M3FN for |v| <= 240; inputs ~N(0,1))
  3. Transpose via PE matmul-against-identity into [d, t] layout (exact), ACT
     copies PSUM f32 -> SBUF fp8 (exact: values are e4m3-representable)
  4. fp8 DoubleRow matmuls (K=256 per step) accumulate in PSUM f32
  5. DVE tensor_add applies bias (f32) while evicting PSUM -> SBUF
  6. DMA out block [128, 4096] f32 -> DRAM
"""

import os
import sys

for _p in ("/opt/trn_rl_repo", "/opt/pypackages"):
    if os.path.isdir(_p) and _p not in sys.path:
        sys.path.append(_p)

from contextlib import ExitStack

import numpy as np

import concourse.bass as bass
import concourse.mybir as mybir
import concourse.tile as tile
from concourse import bacc
from concourse.bass_utils import run_bass_kernel_spmd

P = 128
D_MODEL = 1024
UNITS = 4096
TOKENS = 32768
N_CORES = 8
TPC = TOKENS // N_CORES  # tokens per core
N_FREE = 512  # psum bank free dim (f32)
F32 = mybir.dt.float32
FP8 = mybir.dt.float8e4
U16 = mybir.dt.uint16

KS = D_MODEL // P  # 8 k-subtiles of 128
NU = UNITS // N_FREE  # 8 u-tiles of 512


def build_nc(tpc: int = TPC) -> bass.Bass:
    TB = tpc // P  # token blocks per core

    # Bacc (not plain Bass): its finalize runs generate_event_semaphores,
    # which splits multi-wait instructions — walrus allows only 1 wait/inst.
    nc = bacc.Bacc(
        "TRN2",
        target_bir_lowering=False,
        debug=False,
        enable_asserts=False,
        num_devices=N_CORES,
    )
    x_d = nc.declare_dram_parameter("x", [tpc, D_MODEL], F32, isOutput=False)
    w_d = nc.declare_dram_parameter("w", [D_MODEL, UNITS], F32, isOutput=False)
    b_d = nc.declare_dram_parameter("b", [P, UNITS], F32, isOutput=False)
    o_d = nc.declare_dram_parameter("out", [tpc, UNITS], F32, isOutput=True)

    # Pair-interleaved layout matching the u16 DMA-transpose of x (below):
    # subtile s' = 2k + j holds W rows d = 256k + 2p + j at partition p.
    w_view = w_d[:].rearrange("(k p j) u -> p k j u", p=P, j=2)

    with ExitStack() as ctx:
        tc = ctx.enter_context(tile.TileContext(nc))

        const = ctx.enter_context(tc.tile_pool(name="const", bufs=1))
        bias_sb = const.tile([P, UNITS], F32)
        nc.sync.dma_start(bias_sb[:], b_d[:])

        xqp = ctx.enter_context(tc.tile_pool(name="xq", bufs=4))
        xtp = ctx.enter_context(tc.tile_pool(name="xT", bufs=12))
        ops = ctx.enter_context(tc.tile_pool(name="opsum", bufs=8, space="PSUM"))
        outp = ctx.enter_context(tc.tile_pool(name="outp", bufs=3))

        # Prefetch the first x tiles BEFORE the W chunks on the SWDGE FIFO so
        # the transpose matmuls can start immediately; W streams in behind
        # them and the k-th main matmuls unblock as chunk k lands.
        XPRE = min(4, TB)
        xq_tiles = []
        for t in range(XPRE):
            xq = xqp.tile([P, D_MODEL], FP8)
            nc.gpsimd.dma_start(xq[:], x_d[t * P : (t + 1) * P, :])
            xq_tiles.append(xq)

        # SWDGE cast-DMAs (f32 DRAM -> fp8 SBUF in one transfer; HW cast is
        # bit-exact RNE, verified vs ml_dtypes). One DMA per k-subtile so the
        # first matmuls can start as soon as chunk 0 lands.
        w_fp8 = const.tile([P, KS, UNITS], FP8)
        for s in range(KS):
            k, j = divmod(s, 2)
            nc.gpsimd.dma_start(w_fp8[:, s : s + 1, :], w_view[:, k, j : j + 1, :])

        for t in range(TB):
            if t < XPRE:
                xq = xq_tiles[t]
            else:
                xq = xqp.tile([P, D_MODEL], FP8)
                nc.gpsimd.dma_start(xq[:], x_d[t * P : (t + 1) * P, :])

            # Transpose x via the DMA xbar on u16 views: each u16 element is a
            # (d=2q, d=2q+1) fp8 pair, so transposing [128t, 128q] u16 yields
            # partition q holding that d-pair interleaved along t. Subtile k
            # covers d in [256k, 256k+256); fp8 view [q, (t j)] -> lhsT AP
            # [q, j, t] (t stride 2) matches w_view's s' = 2k + j layout.
            xq16 = xq[:].bitcast(U16)
            xts = []
            for k in range(KS // 2):
                xt16 = xtp.tile([P, P], U16)
                nc.sync.dma_start(xt16[:], xq16[:, k * P : (k + 1) * P], transpose=True)
                xts.append(xt16[:].bitcast(FP8).rearrange("p (t j) -> p j t", j=2))

            ob = outp.tile([P, UNITS], F32)
            for u in range(NU):
                ps = ops.tile([P, N_FREE], F32)
                # plain fp8 (no DoubleRow): DoubleRow's pair-sum adder loses
                # ~6.5e-5 rel accuracy on HW; plain fp8 accumulation is exact
                for s in range(KS):
                    k, j = divmod(s, 2)
                    nc.tensor.matmul(
                        ps[:],
                        lhsT=xts[k][:, j, :],
                        rhs=w_fp8[:, s, u * N_FREE : (u + 1) * N_FREE],
                        start=(s == 0),
                        stop=(s == KS - 1),
                    )
                nc.vector.tensor_add(
                    ob[:, u * N_FREE : (u + 1) * N_FREE],
                    ps[:],
                    bias_sb[:, u * N_FREE : (u + 1) * N_FREE],
                )
            # HWDGE ring for stores; input cast-DMAs live on the SWDGE ring,
            # so a store waiting on ob cannot head-of-line-block input loads
            nc.sync.dma_start(o_d[t * P : (t + 1) * P, :], ob[:])

    nc.finalize()
    return nc


_NC_CACHE: dict = {}


def _get_nc(tpc: int = TPC) -> bass.Bass:
    if tpc not in _NC_CACHE:
        _NC_CACHE[tpc] = build_nc(tpc)
    return _NC_CACHE[tpc]


def run(x, w, bias, trace: bool = False, **kwargs):
    """Shard, execute on 8 cores, gather. Returns (out, BassKernelResults)."""
    x = np.ascontiguousarray(np.asarray(x, dtype=np.float32))
    w = np.ascontiguousarray(np.asarray(w, dtype=np.float32))
    bias = np.asarray(bias, dtype=np.float32).reshape(UNITS)
    b = np.ascontiguousarray(np.broadcast_to(bias[None, :], (P, UNITS)))

    nc = _get_nc(TPC)
    in_maps = [
        {"x": x[c * TPC : (c + 1) * TPC], "w": w, "b": b} for c in range(N_CORES)
    ]
    res = run_bass_kernel_spmd(
        nc, in_maps, list(range(N_CORES)), trace=trace, **kwargs
    )
    out = np.concatenate([r["out"] for r in res.results], axis=0)
    return out, res


def kernel(x, kernel, bias):  # noqa: A002 - harness-specified parameter names
    out, _ = run(x, kernel, bias)
    return out


# revision 20
# speedup vs baseline: 1.0269x; 1.0269x over previous
"""Trainium2 Bass kernel for fp8 (E4M3) quantized dense layer with bias.

Computes: out = fp8(x) @ fp8(W) + bias
  x: [32768, 1024] f32, W: [1024, 4096] f32, bias: [4096] f32 -> out [32768, 4096] f32

Sharding: data-parallel over tokens (32768/8 = 4096 tokens per core); W and bias
replicated. No collectives needed; per-core outputs concatenate along tokens.

Per-core pipeline (tokens processed in blocks of 128):
  1. DMA x block [128, 1024] f32 -> SBUF
  2. ACT cast f32 -> fp8e4 (TRN E4M3 == OCP E4M3FN for |v| <= 240; inputs ~N(0,1))
  3. Transpose via PE matmul-against-identity into [d, t] layout (exact), ACT
     copies PSUM f32 -> SBUF fp8 (exact: values are e4m3-representable)
  4. fp8 DoubleRow matmuls (K=256 per step) accumulate in PSUM f32
  5. DVE tensor_add applies bias (f32) while evicting PSUM -> SBUF
  6. DMA out block [128, 4096] f32 -> DRAM
"""

import os
import sys

for _p in ("/opt/trn_rl_repo", "/opt/pypackages"):
    if os.path.isdir(_p) and _p not in sys.path:
        sys.path.append(_p)

from contextlib import ExitStack

import numpy as np

import concourse.bass as bass
import concourse.mybir as mybir
import concourse.tile as tile
from concourse import bacc
from concourse.bass_utils import run_bass_kernel_spmd

P = 128
D_MODEL = 1024
UNITS = 4096
TOKENS = 32768
N_CORES = 8
TPC = TOKENS // N_CORES  # tokens per core
N_FREE = 512  # psum bank free dim (f32)
F32 = mybir.dt.float32
FP8 = mybir.dt.float8e4
U16 = mybir.dt.uint16

KS = D_MODEL // P  # 8 k-subtiles of 128
NU = UNITS // N_FREE  # 8 u-tiles of 512


def build_nc(tpc: int = TPC) -> bass.Bass:
    TB = tpc // P  # token blocks per core

    # Bacc (not plain Bass): its finalize runs generate_event_semaphores,
    # which splits multi-wait instructions — walrus allows only 1 wait/inst.
    nc = bacc.Bacc(
        "TRN2",
        target_bir_lowering=False,
        debug=False,
        enable_asserts=False,
        num_devices=N_CORES,
    )
    x_d = nc.declare_dram_parameter("x", [tpc, D_MODEL], F32, isOutput=False)
    w_d = nc.declare_dram_parameter("w", [D_MODEL, UNITS], F32, isOutput=False)
    b_d = nc.declare_dram_parameter("b", [P, UNITS], F32, isOutput=False)
    o_d = nc.declare_dram_parameter("out", [tpc, UNITS], F32, isOutput=True)

    # Pair-interleaved layout matching the u16 DMA-transpose of x (below):
    # subtile s' = 2k + j holds W rows d = 256k + 2p + j at partition p.
    w_view = w_d[:].rearrange("(k p j) u -> p k j u", p=P, j=2)

    with ExitStack() as ctx:
        tc = ctx.enter_context(tile.TileContext(nc))

        const = ctx.enter_context(tc.tile_pool(name="const", bufs=1))
        # ACT's HWDGE ring (qActDynamicHW): bias + output stores. The SP ring
        # carries only the u16 DMA-transposes so its xbar stays in transpose
        # mode (mode flips between DMACopy and DMATranspose serialize).
        bias_sb = const.tile([P, UNITS], F32)
        nc.scalar.dma_start(bias_sb[:], b_d[:])

        xqp = ctx.enter_context(tc.tile_pool(name="xq", bufs=4))
        xtp = ctx.enter_context(tc.tile_pool(name="xT", bufs=12))
        ops = ctx.enter_context(tc.tile_pool(name="opsum", bufs=8, space="PSUM"))
        outp = ctx.enter_context(tc.tile_pool(name="outp", bufs=3))

        # Prefetch the first x tiles BEFORE the W chunks on the SWDGE FIFO so
        # the transpose matmuls can start immediately; W streams in behind
        # them and the k-th main matmuls unblock as chunk k lands.
        XPRE = min(4, TB)
        xq_tiles = []
        for t in range(XPRE):
            xq = xqp.tile([P, D_MODEL], FP8)
            nc.gpsimd.dma_start(xq[:], x_d[t * P : (t + 1) * P, :])
            xq_tiles.append(xq)

        # SWDGE cast-DMAs (f32 DRAM -> fp8 SBUF in one transfer; HW cast is
        # bit-exact RNE, verified vs ml_dtypes). One DMA per k-subtile so the
        # first matmuls can start as soon as chunk 0 lands.
        w_fp8 = const.tile([P, KS, UNITS], FP8)
        for s in range(KS):
            k, j = divmod(s, 2)
            nc.gpsimd.dma_start(w_fp8[:, s : s + 1, :], w_view[:, k, j : j + 1, :])

        for t in range(TB):
            if t < XPRE:
                xq = xq_tiles[t]
            else:
                xq = xqp.tile([P, D_MODEL], FP8)
                nc.gpsimd.dma_start(xq[:], x_d[t * P : (t + 1) * P, :])

            # Transpose x via the DMA xbar on u16 views: each u16 element is a
            # (d=2q, d=2q+1) fp8 pair, so transposing [128t, 128q] u16 yields
            # partition q holding that d-pair interleaved along t. Subtile k
            # covers d in [256k, 256k+256); fp8 view [q, (t j)] -> lhsT AP
            # [q, j, t] (t stride 2) matches w_view's s' = 2k + j layout.
            xq16 = xq[:].bitcast(U16)
            xts = []
            for k in range(KS // 2):
                xt16 = xtp.tile([P, P], U16)
                nc.sync.dma_start(xt16[:], xq16[:, k * P : (k + 1) * P], transpose=True)
                xts.append(xt16[:].bitcast(FP8).rearrange("p (t j) -> p j t", j=2))

            ob = outp.tile([P, UNITS], F32)
            for u in range(NU):
                ps = ops.tile([P, N_FREE], F32)
                # plain fp8 (no DoubleRow): DoubleRow's pair-sum adder loses
                # ~6.5e-5 rel accuracy on HW; plain fp8 accumulation is exact
                for s in range(KS):
                    k, j = divmod(s, 2)
                    nc.tensor.matmul(
                        ps[:],
                        lhsT=xts[k][:, j, :],
                        rhs=w_fp8[:, s, u * N_FREE : (u + 1) * N_FREE],
                        start=(s == 0),
                        stop=(s == KS - 1),
                    )
                nc.vector.tensor_add(
                    ob[:, u * N_FREE : (u + 1) * N_FREE],
                    ps[:],
                    bias_sb[:, u * N_FREE : (u + 1) * N_FREE],
                )
            nc.scalar.dma_start(o_d[t * P : (t + 1) * P, :], ob[:])

    nc.finalize()
    return nc


_NC_CACHE: dict = {}


def _get_nc(tpc: int = TPC) -> bass.Bass:
    if tpc not in _NC_CACHE:
        _NC_CACHE[tpc] = build_nc(tpc)
    return _NC_CACHE[tpc]


def run(x, w, bias, trace: bool = False, **kwargs):
    """Shard, execute on 8 cores, gather. Returns (out, BassKernelResults)."""
    x = np.ascontiguousarray(np.asarray(x, dtype=np.float32))
    w = np.ascontiguousarray(np.asarray(w, dtype=np.float32))
    bias = np.asarray(bias, dtype=np.float32).reshape(UNITS)
    b = np.ascontiguousarray(np.broadcast_to(bias[None, :], (P, UNITS)))

    nc = _get_nc(TPC)
    in_maps = [
        {"x": x[c * TPC : (c + 1) * TPC], "w": w, "b": b} for c in range(N_CORES)
    ]
    res = run_bass_kernel_spmd(
        nc, in_maps, list(range(N_CORES)), trace=trace, **kwargs
    )
    out = np.concatenate([r["out"] for r in res.results], axis=0)
    return out, res


def kernel(x, kernel, bias):  # noqa: A002 - harness-specified parameter names
    out, _ = run(x, kernel, bias)
    return out
